# revision 23
# baseline (speedup 1.0000x reference)
"""GCN (4-layer, shared BN) forward on 8 Trainium2 NeuronCores.

Strategy (node-sharded, graph-parallel):
  - Nodes padded to NPAD = 8*SHARD; rank r owns node rows [r*SHARD, (r+1)*SHARD).
  - Edges assigned to the owner of their dst. Per rank, edges are grouped by
    128-node dst block and encoded as (gather index, one-hot scatter matrix).
  - Per layer: every rank gathers u[src] rows (fp16, 256B) from the
    all-gathered node features with dma_gather, scatter-adds them into its
    own dst blocks via one-hot fp8 matmuls accumulated in PSUM, computes
    BatchNorm stats (all-reduced across ranks), applies BN+ReLU+deg^-1/2
    scaling, and all-gathers the updated features.
  - Epilogue: Linear(D,D) + BN + Linear(D,C) computed feature-major
    (PE transpose + W1/W2 as stationary operands), BN2 stats all-reduced.

The int16 gather-index limit (32767) is handled by splitting each block's
edge list into "lo" (src < NPAD/2) and "hi" halves gathered from offset
views of the node-feature table.
"""

import numpy as np
import ml_dtypes

P = 128
D = 128
L = 4
EPS = 1e-5
NCORES = 8
CHUNK_TILES = 8            # gather tiles (128 idx each) per dma_gather call
NQUEUES = 4

_cache = {}


# ----------------------------------------------------------------------------
# host-side preprocessing
# ----------------------------------------------------------------------------

def _preprocess(src, dst, n_nodes):
    """Edge grouping/padding. Returns per-rank tensors + shared metadata."""
    npad = ((n_nodes + NCORES * P - 1) // (NCORES * P)) * NCORES * P
    shard = npad // NCORES
    nb = shard // P
    half = npad // 2
    assert half <= 32768

    order = np.argsort(dst, kind="stable")
    s_sorted = src[order].astype(np.int64)
    d_sorted = dst[order].astype(np.int64)

    # per (rank, block, lo/hi) edge lists
    # block id global: dst // P in [0, NCORES*nb)
    gblock = d_sorted // P
    dl = d_sorted % P
    # ag_out rows are p-major within each rank shard: node (r, b, p) sits at
    # row r*shard + p*nb + b, so SBUF->DRAM staging is contiguous per partition
    _l = s_sorted % shard
    s_row = (s_sorted // shard) * shard + (_l % P) * nb + (_l // P)
    is_hi = s_row >= half

    # counts per (gblock, side)
    nblocks = NCORES * nb
    cnt = np.zeros((nblocks, 2), np.int64)
    np.add.at(cnt, (gblock, is_hi.astype(np.int64)), 1)
    cnt_r = cnt.reshape(NCORES, nb, 2)
    tiles_needed = (cnt_r + P - 1) // P            # [rank, block, side]
    TL = np.maximum(tiles_needed[:, :, 0].max(axis=0), 1)   # [nb]
    TH = np.maximum(tiles_needed[:, :, 1].max(axis=0), 1)
    TLsum, THsum = int(TL.sum()), int(TH.sum())
    T = TLsum + THsum
    LO_OFF = np.concatenate([[0], np.cumsum(TL)])[:-1]
    HI_OFF = np.concatenate([[0], np.cumsum(TH)])[:-1] + TLsum

    CL = (TLsum + CHUNK_TILES - 1) // CHUNK_TILES  # lo gather chunks
    CH = (THsum + CHUNK_TILES - 1) // CHUNK_TILES

    per_rank = []
    for r in range(NCORES):
        sel = (gblock >= r * nb) & (gblock < (r + 1) * nb)
        sb = gblock[sel] - r * nb      # local block
        ss = s_row[sel]
        sd = dl[sel]
        sh = is_hi[sel]

        idx_lo = np.zeros(TLsum * P, np.int64)
        idx_hi = np.zeros(THsum * P, np.int64)
        m_rows = np.empty(len(ss), np.int64)    # edge slot within tile
        m_cols = np.empty(len(ss), np.int64)    # tile*P + dst_local
        pos = 0
        mpos = 0
        for b in range(nb):
            for side in (0, 1):
                m = (sb == b) & (sh == (side == 1))
                srcs = ss[m]
                dls = sd[m]
                n = len(srcs)
                if side == 0:
                    base_tile = LO_OFF[b]
                    idx_arr = idx_lo
                    arr_off = int(LO_OFF[b]) * P
                    vals = srcs
                else:
                    base_tile = HI_OFF[b] - TLsum
                    idx_arr = idx_hi
                    arr_off = int(base_tile) * P
                    vals = srcs - half
                idx_arr[arr_off:arr_off + n] = vals
                # M one-hot: edge j (within this (b, side) list) sits at
                # tile = base_global_tile + j//P, slot j%P
                if side == 0:
                    gt0 = LO_OFF[b]
                else:
                    gt0 = HI_OFF[b]
                j = np.arange(n)
                m_rows[mpos:mpos + n] = j % P
                m_cols[mpos:mpos + n] = (gt0 + j // P) * P + dls
                mpos += n
        assert mpos == len(ss)

        M = np.zeros((P, T * P), np.float32)
        M[m_rows, m_cols] = 1.0
        M = M.astype(ml_dtypes.float8_e4m3)

        def wrap(flat, n_chunks):
            # pad to full chunks of CHUNK_TILES*P idx, wrap to int16 table
            want = n_chunks * CHUNK_TILES * P
            fl = np.zeros(want, np.int64)
            fl[:len(flat)] = flat
            t16 = fl.reshape(-1, 16).T.astype(np.int16)      # [16, want/16]
            return np.tile(t16, (8, 1)).copy()               # [128, want/16]

        deg = np.zeros(npad, np.float32)
        np.add.at(deg, dst, 1.0)
        degcols = deg[r * shard:(r + 1) * shard].reshape(nb, P).T.copy()

        mask = np.zeros((P, nb), np.float32)
        gidx = r * shard + np.arange(shard)
        mask[:, :] = (gidx.reshape(nb, P).T < n_nodes)

        hpad_rows = shard if (r + 1) * shard <= n_nodes else max(0, n_nodes - r * shard)

        per_rank.append(dict(
            M=np.ascontiguousarray(M),
            idx_lo=wrap(idx_lo, CL),
            idx_hi=wrap(idx_hi, CH),
            mask=mask, h_rows=hpad_rows, deg=degcols,
        ))

    meta = dict(npad=npad, shard=shard, nb=nb, half=half,
                TL=TL, TH=TH, T=T, TLsum=TLsum, THsum=THsum,
                LO_OFF=LO_OFF, HI_OFF=HI_OFF, CL=CL, CH=CH,
                n_nodes=n_nodes)
    return meta, per_rank


# ----------------------------------------------------------------------------
# device kernel
# ----------------------------------------------------------------------------

def _build(meta, c_dim):
    import concourse.bacc as bacc
    import concourse.bass as bass
    import concourse.mybir as mybir
    from concourse._compat import get_trn_type
    from concourse.tile import TileContext
    from concourse.masks import make_identity

    npad, shard, nb, half = meta["npad"], meta["shard"], meta["nb"], meta["half"]
    TL, TH = meta["TL"], meta["TH"]
    T, TLsum, THsum = meta["T"], meta["TLsum"], meta["THsum"]
    LO_OFF, HI_OFF = meta["LO_OFF"], meta["HI_OFF"]
    CL, CH = meta["CL"], meta["CH"]
    n_nodes = meta["n_nodes"]
    padcnt = float(npad - n_nodes)
    f32 = mybir.dt.float32
    f16 = mybir.dt.float16
    fp8 = mybir.dt.float8e4
    i16 = mybir.dt.int16
    AF = mybir.ActivationFunctionType
    OP = mybir.AluOpType
    rg = [list(range(NCORES))]

    nc = bacc.Bacc(get_trn_type() or "TRN2", num_devices=NCORES,
                   num_swdge_queues=NQUEUES)

    # ---- I/O ----
    h_in = nc.dram_tensor("h_in", [shard, D], f32, kind="ExternalInput")
    M_in = nc.dram_tensor("M_in", [P, T * P], fp8, kind="ExternalInput")
    il_in = nc.dram_tensor("il_in", [P, CL * CHUNK_TILES * 8], i16, kind="ExternalInput")
    ih_in = nc.dram_tensor("ih_in", [P, CH * CHUNK_TILES * 8], i16, kind="ExternalInput")
    mask_in = nc.dram_tensor("mask_in", [P, nb], f32, kind="ExternalInput")
    deg_in = nc.dram_tensor("deg_in", [P, nb], f32, kind="ExternalInput")
    gam_in = nc.dram_tensor("gam_in", [1, D], f32, kind="ExternalInput")
    bet_in = nc.dram_tensor("bet_in", [1, D], f32, kind="ExternalInput")
    W1_in = nc.dram_tensor("W1_in", [D, D], f32, kind="ExternalInput")
    b1c_in = nc.dram_tensor("b1c_in", [D, 1], f32, kind="ExternalInput")
    g2c_in = nc.dram_tensor("g2c_in", [D, 1], f32, kind="ExternalInput")
    b2c_in = nc.dram_tensor("b2c_in", [D, 1], f32, kind="ExternalInput")
    W2_in = nc.dram_tensor("W2_in", [D, c_dim], f32, kind="ExternalInput")
    bo_in = nc.dram_tensor("bo_in", [c_dim, 1], f32, kind="ExternalInput")
    yT = nc.dram_tensor("yT", [c_dim, shard], f32, kind="ExternalOutput")

    # ---- internal DRAM (collectives) ----
    ag_in = nc.dram_tensor("ag_in", [shard, D], f16, kind="Internal")
    ag_out = nc.dram_tensor("ag_out", [npad, D], f16, kind="Internal")
    ar_in = nc.dram_tensor("ar_in", [1, 2 * D], f32, kind="Internal")
    ar_out = nc.dram_tensor("ar_out", [1, 2 * D], f32, kind="Internal",
                            addr_space="Shared")
    ar2_in = nc.dram_tensor("ar2_in", [D, 2], f32, kind="Internal")
    ar2_out = nc.dram_tensor("ar2_out", [D, 2], f32, kind="Internal",
                             addr_space="Shared")

    lo_view = ag_out[0:half, :]
    hi_view = ag_out[half:npad, :]

    # lo tile t lives in chunk t//CHUNK_TILES slot t%CHUNK_TILES, likewise hi.
    with TileContext(nc, num_cores=NCORES) as tc:
        with (
            tc.tile_pool(name="persist", bufs=1) as pp,
            tc.tile_pool(name="xp", bufs=10) as xp,
            tc.tile_pool(name="ps", bufs=4, space="PSUM") as psp,
            tc.tile_pool(name="ps2", bufs=1, space="PSUM") as psp2,
            tc.tile_pool(name="psw", bufs=2, space="PSUM") as pspw,
            tc.tile_pool(name="tmp", bufs=2) as tp,
            tc.tile_pool(name="rows", bufs=2) as rp,
        ):
            # ---- persistent tiles ----
            M_s = pp.tile([P, T * P], fp8, tag="M")
            il_s = pp.tile([P, CL * CHUNK_TILES * 8], i16, tag="il")
            ih_s = pp.tile([P, CH * CHUNK_TILES * 8], i16, tag="ih")
            s_all = pp.tile([P, nb, D], f32, tag="s")
            u_all = pp.tile([P, nb, D], f16, tag="u")
            norm = pp.tile([P, nb], f32, tag="norm")
            mask_s = pp.tile([P, nb], f32, tag="mask")
            a_full = pp.tile([P, D], f32, tag="af")
            b_full = pp.tile([P, D], f32, tag="bf")
            ones16 = pp.tile([P, 1], f16, tag="o16")
            ones_l = pp.tile([P, 1], f32, tag="ol")
            ones_row = pp.tile([1, P], f32, tag="orow")
            ident = pp.tile([P, P], f32, tag="ident")
            W1_s = pp.tile([D, D], f32, tag="W1")
            W2_s = pp.tile([D, c_dim], f32, tag="W2")
            b1c = pp.tile([D, 1], f32, tag="b1c")
            g2c = pp.tile([D, 1], f32, tag="g2c")
            b2c = pp.tile([D, 1], f32, tag="b2c")
            boc = pp.tile([c_dim, 1], f32, tag="boc")
            gam = pp.tile([1, D], f32, tag="gam")
            bet = pp.tile([1, D], f32, tag="bet")
            xT_all = pp.tile([P, nb, D], f32, tag="xT")
            s1m = pp.tile([P, nb], f32, tag="s1m")
            s2m = pp.tile([P, nb], f32, tag="s2m")

            nc.vector.memset(ones16[:], 1.0)
            nc.vector.memset(ones_l[:], 1.0)
            nc.vector.memset(ones_row[:], 1.0)
            make_identity(nc, ident[:])

            nc.sync.dma_start(mask_s[:], mask_in[:])
            nc.sync.dma_start(W1_s[:], W1_in[:])
            nc.sync.dma_start(W2_s[:], W2_in[:])
            nc.sync.dma_start(b1c[:], b1c_in[:])
            nc.sync.dma_start(g2c[:], g2c_in[:])
            nc.sync.dma_start(b2c[:], b2c_in[:])
            nc.sync.dma_start(boc[:], bo_in[:])
            nc.sync.dma_start(gam[:], gam_in[:])
            nc.sync.dma_start(bet[:], bet_in[:])

            def block_tiles(b):
                lo = [int(LO_OFF[b]) + t for t in range(int(TL[b]))]
                hi = [int(HI_OFF[b]) + t for t in range(int(TH[b]))]
                return lo, hi

            # ---- prologue: norm = rsqrt(max(deg, 1)) ----
            degt = tp.tile([P, nb], f32, tag="degt")
            nc.sync.dma_start(degt[:], deg_in[:])
            nc.vector.tensor_scalar_max(degt[:], degt[:], 1.0)
            nc.scalar.activation(degt[:], degt[:], AF.Sqrt)
            nc.vector.reciprocal(norm[:], degt[:])

            # ---- u0 = h * norm ----
            h_all = s_all  # reuse the s buffer to hold h tiles
            nc.sync.dma_start(
                h_all[:], h_in[:].rearrange("(b p) d -> p b d", p=P))
            nc.vector.tensor_tensor(
                u_all[:], h_all[:],
                norm[:, :, None].to_broadcast([P, nb, D]), op=OP.mult)
            nc.sync.dma_start(
                ag_in[:].rearrange("(p b) d -> p b d", p=P), u_all[:])
            nc.gpsimd.collective_compute(
                "AllGather", OP.bypass, ins=[ag_in[:].opt()],
                outs=[ag_out[:].opt()], replica_groups=rg)

            nc.sync.dma_start(M_s[:], M_in[:])
            nc.sync.dma_start(il_s[:], il_in[:])
            nc.sync.dma_start(ih_s[:], ih_in[:])

            # ---- layers ----
            for lyr in range(L):
                s1ps = psp2.tile([1, D], f32, tag="s1", space="PSUM")
                s2ps = psp2.tile([1, D], f32, tag="s2", space="PSUM")

                # -- lo pass --
                lo_chunks = [None] * CL
                hi_chunks = [None] * CH

                def gather_chunk(c, region):
                    tbl, view, nchunks = (
                        (il_s, lo_view, CL) if region == 0 else (ih_s, hi_view, CH))
                    xt = xp.tile([P, CHUNK_TILES, D], f16, tag="x")
                    nidx = CHUNK_TILES * P
                    nc.gpsimd.dma_gather(
                        xt[:], view, tbl[:, c * CHUNK_TILES * 8:(c + 1) * CHUNK_TILES * 8],
                        nidx, nidx, D, single_packet=True,
                        queue_num=(c % NQUEUES))
                    return xt

                # issue all lo gathers lazily as consumed; Tile's scheduler
                # overlaps them via the pool's 6 slots.
                for b in range(nb):
                    lo, hi = block_tiles(b)
                    sps = psp.tile([P, D], f32, tag="ps", space="PSUM")
                    for i, t in enumerate(lo):
                        c, slot = t // CHUNK_TILES, t % CHUNK_TILES
                        if lo_chunks[c] is None:
                            lo_chunks[c] = gather_chunk(c, 0)
                        nc.tensor.matmul(sps[:], M_s[:, t * P:(t + 1) * P],
                                         lo_chunks[c][:, slot, :],
                                         start=(i == 0), stop=(i == len(lo) - 1))
                    nc.vector.tensor_scalar_mul(s_all[:, b, :], sps[:], norm[:, b:b + 1])
                # -- hi pass --
                for b in range(nb):
                    lo, hi = block_tiles(b)
                    sps = psp.tile([P, D], f32, tag="ps", space="PSUM")
                    for i, t in enumerate(hi):
                        th = t - TLsum
                        c, slot = th // CHUNK_TILES, th % CHUNK_TILES
                        if hi_chunks[c] is None:
                            hi_chunks[c] = gather_chunk(c, 1)
                        nc.tensor.matmul(sps[:], M_s[:, t * P:(t + 1) * P],
                                         hi_chunks[c][:, slot, :],
                                         start=(i == 0), stop=(i == len(hi) - 1))
                    nc.vector.scalar_tensor_tensor(
                        s_all[:, b, :], sps[:], norm[:, b:b + 1], s_all[:, b, :],
                        op0=OP.mult, op1=OP.add)
                    # stats contributions
                    sq = tp.tile([P, D], f32, tag="sq")
                    nc.scalar.activation(sq[:], s_all[:, b, :], AF.Square)
                    nc.tensor.matmul(s1ps[:], ones_l[:], s_all[:, b, :],
                                     start=(b == 0), stop=(b == nb - 1))
                    nc.tensor.matmul(s2ps[:], ones_l[:], sq[:],
                                     start=(b == 0), stop=(b == nb - 1))

                # -- stats allreduce --
                arst = rp.tile([1, 2 * D], f32, tag="arst")
                nc.vector.tensor_copy(arst[:, 0:D], s1ps[:])
                nc.vector.tensor_copy(arst[:, D:2 * D], s2ps[:])
                nc.sync.dma_start(ar_in[:], arst[:])
                nc.gpsimd.collective_compute(
                    "AllReduce", OP.add, ins=[ar_in[:].opt()],
                    outs=[ar_out[:].opt()], replica_groups=rg)
                stat = rp.tile([1, 2 * D], f32, tag="stat")
                nc.sync.dma_start(stat[:], ar_out[:])

                # -- BN coefficients (rows on partition 0) --
                mu = rp.tile([1, D], f32, tag="mu")
                var = rp.tile([1, D], f32, tag="var")
                rs = rp.tile([1, D], f32, tag="rs")
                arow = rp.tile([1, D], f32, tag="arow")
                brow = rp.tile([1, D], f32, tag="brow")
                nc.vector.tensor_scalar_mul(mu[:], stat[:, 0:D], 1.0 / n_nodes)
                nc.vector.tensor_scalar_mul(var[:], stat[:, D:2 * D], 1.0 / n_nodes)
                tmp = rp.tile([1, D], f32, tag="musq")
                nc.vector.tensor_tensor(tmp[:], mu[:], mu[:], op=OP.mult)
                nc.vector.tensor_tensor(var[:], var[:], tmp[:], op=OP.subtract)
                nc.vector.tensor_scalar_add(var[:], var[:], EPS)
                nc.scalar.activation(rs[:], var[:], AF.Sqrt)
                nc.vector.reciprocal(rs[:], rs[:])
                nc.vector.tensor_tensor(arow[:], gam[:], rs[:], op=OP.mult)
                nc.vector.tensor_tensor(tmp[:], mu[:], arow[:], op=OP.mult)
                nc.vector.tensor_tensor(brow[:], bet[:], tmp[:], op=OP.subtract)
                afps = psp.tile([P, D], f32, tag="ps", space="PSUM")
                nc.tensor.matmul(afps[:], ones_row[:], arow[:], start=True, stop=True)
                nc.vector.tensor_copy(a_full[:], afps[:])
                bfps = psp.tile([P, D], f32, tag="ps", space="PSUM")
                nc.tensor.matmul(bfps[:], ones_row[:], brow[:], start=True, stop=True)
                nc.vector.tensor_copy(b_full[:], bfps[:])

                # -- apply BN + relu (+ norm or mask) --
                arep = a_full[:, None, :].to_broadcast([P, nb, D])
                brep = b_full[:, None, :].to_broadcast([P, nb, D])
                if lyr < L - 1:
                    nc.vector.tensor_tensor(s_all[:], s_all[:], arep, op=OP.mult)
                    nc.vector.tensor_tensor(s_all[:], s_all[:], brep, op=OP.add)
                    nc.vector.scalar_tensor_tensor(
                        u_all[:], s_all[:], 0.0,
                        norm[:, :, None].to_broadcast([P, nb, D]),
                        op0=OP.max, op1=OP.mult)
                    nc.sync.dma_start(
                        ag_in[:].rearrange("(p b) d -> p b d", p=P), u_all[:])
                    nc.gpsimd.collective_compute(
                        "AllGather", OP.bypass, ins=[ag_in[:].opt()],
                        outs=[ag_out[:].opt()], replica_groups=rg)
                else:
                    nc.vector.tensor_tensor(s_all[:], s_all[:], arep, op=OP.mult)
                    nc.vector.tensor_tensor(s_all[:], s_all[:], brep, op=OP.add)
                    nc.vector.scalar_tensor_tensor(
                        s_all[:], s_all[:], 0.0,
                        mask_s[:, :, None].to_broadcast([P, nb, D]),
                        op0=OP.max, op1=OP.mult)

            # ---- epilogue: x = h4 @ W1 + b1 (feature-major), BN2, W2 ----
            GRP = 4
            for g in range(0, nb, GRP):
                gw = min(GRP, nb - g)
                tr4 = tp.tile([P, GRP * D], f32, tag="tr4")
                for k in range(gw):
                    trp = psp.tile([P, D], f32, tag="ps", space="PSUM")
                    nc.tensor.transpose(out=trp[:], in_=s_all[:, g + k, :],
                                        identity=ident[:])
                    nc.vector.tensor_copy(tr4[:, k * D:(k + 1) * D], trp[:])
                xps = pspw.tile([P, GRP * D], f32, tag="psw", space="PSUM")
                nc.tensor.matmul(xps[:, :gw * D], W1_s[:], tr4[:, :gw * D],
                                 start=True, stop=True)
                nc.vector.tensor_scalar_add(
                    xT_all[:, g:g + gw, :].rearrange("p b d -> p (b d)"),
                    xps[:, :gw * D], b1c[:])
            st2 = rp.tile([D, 2], f32, tag="st2")
            nc.vector.tensor_reduce(st2[:, 0:1], xT_all[:],
                                    axis=mybir.AxisListType.XY, op=OP.add)
            nc.scalar.activation(s_all[:], xT_all[:], AF.Square)
            nc.vector.tensor_reduce(st2[:, 1:2], s_all[:],
                                    axis=mybir.AxisListType.XY, op=OP.add)
            nc.sync.dma_start(ar2_in[:], st2[:])
            nc.gpsimd.collective_compute(
                "AllReduce", OP.add, ins=[ar2_in[:].opt()],
                outs=[ar2_out[:].opt()], replica_groups=rg)
            st2g = rp.tile([D, 2], f32, tag="st2g")
            nc.sync.dma_start(st2g[:], ar2_out[:])

            # pad-row correction: pad nodes contribute b1 to S1 and b1^2 to S2
            s1c = rp.tile([D, 1], f32, tag="s1c")
            s2c = rp.tile([D, 1], f32, tag="s2c")
            b1sq = rp.tile([D, 1], f32, tag="b1sq")
            nc.vector.scalar_tensor_tensor(s1c[:], b1c[:], -padcnt, st2g[:, 0:1],
                                           op0=OP.mult, op1=OP.add)
            nc.scalar.activation(b1sq[:], b1c[:], AF.Square)
            nc.vector.scalar_tensor_tensor(s2c[:], b1sq[:], -padcnt, st2g[:, 1:2],
                                           op0=OP.mult, op1=OP.add)

            mu2 = rp.tile([D, 1], f32, tag="mu2")
            var2 = rp.tile([D, 1], f32, tag="var2")
            rs2 = rp.tile([D, 1], f32, tag="rs2")
            a2 = rp.tile([D, 1], f32, tag="a2")
            bb2 = rp.tile([D, 1], f32, tag="bb2")
            t2 = rp.tile([D, 1], f32, tag="t2")
            nc.vector.tensor_scalar_mul(mu2[:], s1c[:], 1.0 / n_nodes)
            nc.vector.tensor_scalar_mul(var2[:], s2c[:], 1.0 / n_nodes)
            nc.vector.tensor_tensor(t2[:], mu2[:], mu2[:], op=OP.mult)
            nc.vector.tensor_tensor(var2[:], var2[:], t2[:], op=OP.subtract)
            nc.vector.tensor_scalar_add(var2[:], var2[:], EPS)
            nc.scalar.activation(rs2[:], var2[:], AF.Sqrt)
            nc.vector.reciprocal(rs2[:], rs2[:])
            nc.vector.tensor_tensor(a2[:], g2c[:], rs2[:], op=OP.mult)
            nc.vector.tensor_tensor(t2[:], mu2[:], a2[:], op=OP.mult)
            nc.vector.tensor_tensor(bb2[:], b2c[:], t2[:], op=OP.subtract)

            # xT_bn = xT * a2 + bb2 (per-partition scalars), then W2
            nc.vector.tensor_scalar(xT_all[:], xT_all[:], a2[:], bb2[:],
                                    op0=OP.mult, op1=OP.add)
            for g in range(0, nb, GRP):
                gw = min(GRP, nb - g)
                yps = pspw.tile([c_dim, GRP * D], f32, tag="psw", space="PSUM")
                nc.tensor.matmul(yps[:, :gw * D], W2_s[:],
                                 xT_all[:, g:g + gw, :].rearrange("p b d -> p (b d)"),
                                 start=True, stop=True)
                yst = tp.tile([c_dim, GRP * D], f32, tag="yst")
                nc.vector.tensor_scalar_add(yst[:, :gw * D], yps[:, :gw * D], boc[:])
                nc.sync.dma_start(yT[:, g * D:(g + gw) * D], yst[:, :gw * D])

    nc.compile()
    return nc


# ----------------------------------------------------------------------------
# entry point
# ----------------------------------------------------------------------------

def kernel(h, src, dst, bn_gamma, bn_beta, W1, b1, bn2_gamma, bn2_beta, W2, b2,
           _return_perf=None):
    from concourse.bass_utils import run_bass_kernel_spmd

    h = np.asarray(h, np.float32)
    src = np.asarray(src)
    dst = np.asarray(dst)
    n_nodes = h.shape[0]
    c_dim = np.asarray(W2).shape[1]

    key = (n_nodes, src.tobytes()[:4096], dst.tobytes()[:4096], len(src))
    if key in _cache:
        meta, per_rank, nc = _cache[key]
    else:
        meta, per_rank = _preprocess(src, dst, n_nodes)
        nc = _build(meta, c_dim)
        _cache.clear()
        _cache[key] = (meta, per_rank, nc)

    shard = meta["shard"]
    in_maps = []
    for r in range(NCORES):
        pr = per_rank[r]
        hsh = np.zeros((shard, D), np.float32)
        nrows = pr["h_rows"]
        if nrows > 0:
            hsh[:nrows] = h[r * shard: r * shard + nrows]
        in_maps.append({
            "h_in": hsh,
            "M_in": pr["M"],
            "il_in": pr["idx_lo"],
            "ih_in": pr["idx_hi"],
            "mask_in": pr["mask"],
            "deg_in": pr["deg"],
            "gam_in": np.asarray(bn_gamma, np.float32).reshape(1, D),
            "bet_in": np.asarray(bn_beta, np.float32).reshape(1, D),
            "W1_in": np.asarray(W1, np.float32),
            "b1c_in": np.asarray(b1, np.float32).reshape(D, 1),
            "g2c_in": np.asarray(bn2_gamma, np.float32).reshape(D, 1),
            "b2c_in": np.asarray(bn2_beta, np.float32).reshape(D, 1),
            "W2_in": np.asarray(W2, np.float32),
            "bo_in": np.asarray(b2, np.float32).reshape(c_dim, 1),
        })

    kw = {}
    if _return_perf:
        kw = dict(trace=True, tmpdir=_return_perf)
    res = run_bass_kernel_spmd(nc, in_maps, core_ids=list(range(NCORES)), **kw)

    out = np.empty((n_nodes, c_dim), np.float32)
    for r in range(NCORES):
        nrows = min(shard, max(0, n_nodes - r * shard))
        if nrows > 0:
            out[r * shard: r * shard + nrows] = res.results[r]["yT"].T[:nrows]
    if _return_perf:
        kernel.last_exec_time_ns = res.exec_time_ns
    return out



# revision 25
# speedup vs baseline: 1.0616x; 1.0616x over previous
"""GCN (4-layer, shared BN) forward on 8 Trainium2 NeuronCores.

Strategy (node-sharded, graph-parallel):
  - Nodes padded to NPAD = 8*SHARD; rank r owns node rows [r*SHARD, (r+1)*SHARD).
  - Edges assigned to the owner of their dst. Per rank, edges are grouped by
    128-node dst block and encoded as (gather index, one-hot scatter matrix).
  - Per layer: every rank gathers u[src] rows (fp16, 256B) from the
    all-gathered node features with dma_gather, scatter-adds them into its
    own dst blocks via one-hot fp8 matmuls accumulated in PSUM, computes
    BatchNorm stats (all-reduced across ranks), applies BN+ReLU+deg^-1/2
    scaling, and all-gathers the updated features.
  - Epilogue: Linear(D,D) + BN + Linear(D,C) computed feature-major
    (PE transpose + W1/W2 as stationary operands), BN2 stats all-reduced.

The int16 gather-index limit (32767) is handled by splitting each block's
edge list into "lo" (src < NPAD/2) and "hi" halves gathered from offset
views of the node-feature table.
"""

import numpy as np
import ml_dtypes

P = 128
D = 128
L = 4
EPS = 1e-5
NCORES = 8
CHUNK_TILES = 8            # gather tiles (128 idx each) per dma_gather call
NQUEUES = 4

_cache = {}


# ----------------------------------------------------------------------------
# host-side preprocessing
# ----------------------------------------------------------------------------

def _preprocess(src, dst, n_nodes):
    """Edge grouping/padding. Returns per-rank tensors + shared metadata."""
    npad = ((n_nodes + NCORES * P - 1) // (NCORES * P)) * NCORES * P
    shard = npad // NCORES
    nb = shard // P
    half = npad // 2
    assert half <= 32768

    order = np.argsort(dst, kind="stable")
    s_sorted = src[order].astype(np.int64)
    d_sorted = dst[order].astype(np.int64)

    # per (rank, block, lo/hi) edge lists
    # block id global: dst // P in [0, NCORES*nb)
    gblock = d_sorted // P
    dl = d_sorted % P
    # ag_out rows are p-major within each rank shard: node (r, b, p) sits at
    # row r*shard + p*nb + b, so SBUF->DRAM staging is contiguous per partition
    _l = s_sorted % shard
    s_row = (s_sorted // shard) * shard + (_l % P) * nb + (_l // P)
    is_hi = s_row >= half

    # counts per (gblock, side)
    nblocks = NCORES * nb
    cnt = np.zeros((nblocks, 2), np.int64)
    np.add.at(cnt, (gblock, is_hi.astype(np.int64)), 1)
    cnt_r = cnt.reshape(NCORES, nb, 2)
    tiles_needed = (cnt_r + P - 1) // P            # [rank, block, side]
    TL = np.maximum(tiles_needed[:, :, 0].max(axis=0), 1)   # [nb]
    TH = np.maximum(tiles_needed[:, :, 1].max(axis=0), 1)
    TLsum, THsum = int(TL.sum()), int(TH.sum())
    T = TLsum + THsum
    LO_OFF = np.concatenate([[0], np.cumsum(TL)])[:-1]
    HI_OFF = np.concatenate([[0], np.cumsum(TH)])[:-1] + TLsum

    CL = (TLsum + CHUNK_TILES - 1) // CHUNK_TILES  # lo gather chunks
    CH = (THsum + CHUNK_TILES - 1) // CHUNK_TILES

    per_rank = []
    for r in range(NCORES):
        sel = (gblock >= r * nb) & (gblock < (r + 1) * nb)
        sb = gblock[sel] - r * nb      # local block
        ss = s_row[sel]
        sd = dl[sel]
        sh = is_hi[sel]

        idx_lo = np.zeros(TLsum * P, np.int64)
        idx_hi = np.zeros(THsum * P, np.int64)
        m_rows = np.empty(len(ss), np.int64)    # edge slot within tile
        m_cols = np.empty(len(ss), np.int64)    # tile*P + dst_local
        pos = 0
        mpos = 0
        for b in range(nb):
            for side in (0, 1):
                m = (sb == b) & (sh == (side == 1))
                srcs = ss[m]
                dls = sd[m]
                n = len(srcs)
                if side == 0:
                    base_tile = LO_OFF[b]
                    idx_arr = idx_lo
                    arr_off = int(LO_OFF[b]) * P
                    vals = srcs
                else:
                    base_tile = HI_OFF[b] - TLsum
                    idx_arr = idx_hi
                    arr_off = int(base_tile) * P
                    vals = srcs - half
                idx_arr[arr_off:arr_off + n] = vals
                # M one-hot: edge j (within this (b, side) list) sits at
                # tile = base_global_tile + j//P, slot j%P
                if side == 0:
                    gt0 = LO_OFF[b]
                else:
                    gt0 = HI_OFF[b]
                j = np.arange(n)
                m_rows[mpos:mpos + n] = j % P
                m_cols[mpos:mpos + n] = (gt0 + j // P) * P + dls
                mpos += n
        assert mpos == len(ss)

        M = np.zeros((P, T * P), np.float32)
        M[m_rows, m_cols] = 1.0
        M = M.astype(ml_dtypes.float8_e4m3)

        def wrap(flat, n_chunks):
            # pad to full chunks of CHUNK_TILES*P idx, wrap to int16 table
            want = n_chunks * CHUNK_TILES * P
            fl = np.zeros(want, np.int64)
            fl[:len(flat)] = flat
            t16 = fl.reshape(-1, 16).T.astype(np.int16)      # [16, want/16]
            return np.tile(t16, (8, 1)).copy()               # [128, want/16]

        deg = np.zeros(npad, np.float32)
        np.add.at(deg, dst, 1.0)
        degcols = deg[r * shard:(r + 1) * shard].reshape(nb, P).T.copy()

        mask = np.zeros((P, nb), np.float32)
        gidx = r * shard + np.arange(shard)
        mask[:, :] = (gidx.reshape(nb, P).T < n_nodes)

        hpad_rows = shard if (r + 1) * shard <= n_nodes else max(0, n_nodes - r * shard)

        per_rank.append(dict(
            M=np.ascontiguousarray(M),
            idx_lo=wrap(idx_lo, CL),
            idx_hi=wrap(idx_hi, CH),
            mask=mask, h_rows=hpad_rows, deg=degcols,
        ))

    meta = dict(npad=npad, shard=shard, nb=nb, half=half,
                TL=TL, TH=TH, T=T, TLsum=TLsum, THsum=THsum,
                LO_OFF=LO_OFF, HI_OFF=HI_OFF, CL=CL, CH=CH,
                n_nodes=n_nodes)
    return meta, per_rank


# ----------------------------------------------------------------------------
# device kernel
# ----------------------------------------------------------------------------

def _build(meta, c_dim):
    import concourse.bacc as bacc
    import concourse.bass as bass
    import concourse.mybir as mybir
    from concourse._compat import get_trn_type
    from concourse.tile import TileContext
    from concourse.masks import make_identity

    npad, shard, nb, half = meta["npad"], meta["shard"], meta["nb"], meta["half"]
    TL, TH = meta["TL"], meta["TH"]
    T, TLsum, THsum = meta["T"], meta["TLsum"], meta["THsum"]
    LO_OFF, HI_OFF = meta["LO_OFF"], meta["HI_OFF"]
    CL, CH = meta["CL"], meta["CH"]
    n_nodes = meta["n_nodes"]
    padcnt = float(npad - n_nodes)
    f32 = mybir.dt.float32
    f16 = mybir.dt.float16
    fp8 = mybir.dt.float8e4
    i16 = mybir.dt.int16
    AF = mybir.ActivationFunctionType
    OP = mybir.AluOpType
    rg = [list(range(NCORES))]

    nc = bacc.Bacc(get_trn_type() or "TRN2", num_devices=NCORES,
                   num_swdge_queues=NQUEUES)

    # ---- I/O ----
    h_in = nc.dram_tensor("h_in", [shard, D], f32, kind="ExternalInput")
    M_in = nc.dram_tensor("M_in", [P, T * P], fp8, kind="ExternalInput")
    il_in = nc.dram_tensor("il_in", [P, CL * CHUNK_TILES * 8], i16, kind="ExternalInput")
    ih_in = nc.dram_tensor("ih_in", [P, CH * CHUNK_TILES * 8], i16, kind="ExternalInput")
    mask_in = nc.dram_tensor("mask_in", [P, nb], f32, kind="ExternalInput")
    deg_in = nc.dram_tensor("deg_in", [P, nb], f32, kind="ExternalInput")
    gam_in = nc.dram_tensor("gam_in", [1, D], f32, kind="ExternalInput")
    bet_in = nc.dram_tensor("bet_in", [1, D], f32, kind="ExternalInput")
    W1_in = nc.dram_tensor("W1_in", [D, D], f32, kind="ExternalInput")
    b1c_in = nc.dram_tensor("b1c_in", [D, 1], f32, kind="ExternalInput")
    g2c_in = nc.dram_tensor("g2c_in", [D, 1], f32, kind="ExternalInput")
    b2c_in = nc.dram_tensor("b2c_in", [D, 1], f32, kind="ExternalInput")
    W2_in = nc.dram_tensor("W2_in", [D, c_dim], f32, kind="ExternalInput")
    bo_in = nc.dram_tensor("bo_in", [c_dim, 1], f32, kind="ExternalInput")
    yT = nc.dram_tensor("yT", [c_dim, shard], f32, kind="ExternalOutput")

    # ---- internal DRAM (collectives) ----
    ag_in = nc.dram_tensor("ag_in", [shard, D], f16, kind="Internal")
    ag_out = nc.dram_tensor("ag_out", [npad, D], f16, kind="Internal")
    ar_in = nc.dram_tensor("ar_in", [1, 2 * D], f32, kind="Internal")
    ar_out = nc.dram_tensor("ar_out", [1, 2 * D], f32, kind="Internal",
                            addr_space="Shared")
    ar2_in = nc.dram_tensor("ar2_in", [D, 2], f32, kind="Internal")
    ar2_out = nc.dram_tensor("ar2_out", [D, 2], f32, kind="Internal",
                             addr_space="Shared")

    lo_view = ag_out[0:half, :]
    hi_view = ag_out[half:npad, :]

    # lo tile t lives in chunk t//CHUNK_TILES slot t%CHUNK_TILES, likewise hi.
    with TileContext(nc, num_cores=NCORES) as tc:
        with (
            tc.tile_pool(name="persist", bufs=1) as pp,
            tc.tile_pool(name="xp", bufs=12) as xp,
            tc.tile_pool(name="ps", bufs=4, space="PSUM") as psp,
            tc.tile_pool(name="ps2", bufs=1, space="PSUM") as psp2,
            tc.tile_pool(name="psw", bufs=2, space="PSUM") as pspw,
            tc.tile_pool(name="tmp", bufs=2) as tp,
            tc.tile_pool(name="rows", bufs=2) as rp,
        ):
            # ---- persistent tiles ----
            M_s = pp.tile([P, T * P], fp8, tag="M")
            il_s = pp.tile([P, CL * CHUNK_TILES * 8], i16, tag="il")
            ih_s = pp.tile([P, CH * CHUNK_TILES * 8], i16, tag="ih")
            s_all = pp.tile([P, nb, D], f32, tag="s")
            u_all = pp.tile([P, nb, D], f16, tag="u")
            norm = pp.tile([P, nb], f32, tag="norm")
            mask_s = pp.tile([P, nb], f32, tag="mask")
            a_full = pp.tile([P, D], f32, tag="af")
            b_full = pp.tile([P, D], f32, tag="bf")
            ones16 = pp.tile([P, 1], f16, tag="o16")
            ones_l = pp.tile([P, 1], f32, tag="ol")
            ones_row = pp.tile([1, P], f32, tag="orow")
            ident = pp.tile([P, P], f32, tag="ident")
            W1_s = pp.tile([D, D], f32, tag="W1")
            W2_s = pp.tile([D, c_dim], f32, tag="W2")
            b1c = pp.tile([D, 1], f32, tag="b1c")
            g2c = pp.tile([D, 1], f32, tag="g2c")
            b2c = pp.tile([D, 1], f32, tag="b2c")
            boc = pp.tile([c_dim, 1], f32, tag="boc")
            gam = pp.tile([1, D], f32, tag="gam")
            bet = pp.tile([1, D], f32, tag="bet")
            xT_all = pp.tile([P, nb, D], f32, tag="xT")
            s1m = pp.tile([P, nb], f32, tag="s1m")
            s2m = pp.tile([P, nb], f32, tag="s2m")

            nc.vector.memset(ones16[:], 1.0)
            nc.vector.memset(ones_l[:], 1.0)
            nc.vector.memset(ones_row[:], 1.0)
            make_identity(nc, ident[:])

            nc.sync.dma_start(mask_s[:], mask_in[:])
            nc.sync.dma_start(W1_s[:], W1_in[:])
            nc.sync.dma_start(W2_s[:], W2_in[:])
            nc.sync.dma_start(b1c[:], b1c_in[:])
            nc.sync.dma_start(g2c[:], g2c_in[:])
            nc.sync.dma_start(b2c[:], b2c_in[:])
            nc.sync.dma_start(boc[:], bo_in[:])
            nc.sync.dma_start(gam[:], gam_in[:])
            nc.sync.dma_start(bet[:], bet_in[:])

            def block_tiles(b):
                lo = [int(LO_OFF[b]) + t for t in range(int(TL[b]))]
                hi = [int(HI_OFF[b]) + t for t in range(int(TH[b]))]
                return lo, hi

            # ---- prologue: norm = rsqrt(max(deg, 1)) ----
            degt = tp.tile([P, nb], f32, tag="degt")
            nc.sync.dma_start(degt[:], deg_in[:])
            nc.vector.tensor_scalar_max(degt[:], degt[:], 1.0)
            nc.scalar.activation(degt[:], degt[:], AF.Sqrt)
            nc.vector.reciprocal(norm[:], degt[:])

            # ---- u0 = h * norm ----
            h_all = s_all  # reuse the s buffer to hold h tiles
            nc.sync.dma_start(
                h_all[:], h_in[:].rearrange("(b p) d -> p b d", p=P))
            nc.vector.tensor_tensor(
                u_all[:], h_all[:],
                norm[:, :, None].to_broadcast([P, nb, D]), op=OP.mult)
            nc.sync.dma_start(
                ag_in[:].rearrange("(p b) d -> p b d", p=P), u_all[:])
            nc.gpsimd.collective_compute(
                "AllGather", OP.bypass, ins=[ag_in[:].opt()],
                outs=[ag_out[:].opt()], replica_groups=rg)

            nc.sync.dma_start(M_s[:], M_in[:])
            nc.sync.dma_start(il_s[:], il_in[:])
            nc.sync.dma_start(ih_s[:], ih_in[:])

            # ---- layers ----
            for lyr in range(L):
                s1ps = psp2.tile([1, D], f32, tag="s1", space="PSUM")
                s2ps = psp2.tile([1, D], f32, tag="s2", space="PSUM")

                # -- lo pass --
                lo_chunks = [None] * CL
                hi_chunks = [None] * CH

                def gather_chunk(c, region):
                    tbl, view, nchunks = (
                        (il_s, lo_view, CL) if region == 0 else (ih_s, hi_view, CH))
                    xt = xp.tile([P, CHUNK_TILES, D], f16, tag="x")
                    nidx = CHUNK_TILES * P
                    nc.gpsimd.dma_gather(
                        xt[:], view, tbl[:, c * CHUNK_TILES * 8:(c + 1) * CHUNK_TILES * 8],
                        nidx, nidx, D, single_packet=True,
                        queue_num=(c % NQUEUES))
                    return xt

                # issue all lo gathers lazily as consumed; Tile's scheduler
                # overlaps them via the pool's 6 slots.
                for b in range(nb):
                    lo, hi = block_tiles(b)
                    sps = psp.tile([P, D], f32, tag="ps", space="PSUM")
                    for i, t in enumerate(lo):
                        c, slot = t // CHUNK_TILES, t % CHUNK_TILES
                        if lo_chunks[c] is None:
                            lo_chunks[c] = gather_chunk(c, 0)
                        nc.tensor.matmul(sps[:], M_s[:, t * P:(t + 1) * P],
                                         lo_chunks[c][:, slot, :],
                                         start=(i == 0), stop=(i == len(lo) - 1))
                    nc.vector.tensor_scalar_mul(s_all[:, b, :], sps[:], norm[:, b:b + 1])
                # -- hi pass --
                for b in range(nb):
                    lo, hi = block_tiles(b)
                    sps = psp.tile([P, D], f32, tag="ps", space="PSUM")
                    for i, t in enumerate(hi):
                        th = t - TLsum
                        c, slot = th // CHUNK_TILES, th % CHUNK_TILES
                        if hi_chunks[c] is None:
                            hi_chunks[c] = gather_chunk(c, 1)
                        nc.tensor.matmul(sps[:], M_s[:, t * P:(t + 1) * P],
                                         hi_chunks[c][:, slot, :],
                                         start=(i == 0), stop=(i == len(hi) - 1))
                    nc.vector.scalar_tensor_tensor(
                        s_all[:, b, :], sps[:], norm[:, b:b + 1], s_all[:, b, :],
                        op0=OP.mult, op1=OP.add)
                    # stats contributions
                    sq = tp.tile([P, D], f32, tag="sq")
                    nc.scalar.activation(sq[:], s_all[:, b, :], AF.Square)
                    nc.tensor.matmul(s1ps[:], ones_l[:], s_all[:, b, :],
                                     start=(b == 0), stop=(b == nb - 1))
                    nc.tensor.matmul(s2ps[:], ones_l[:], sq[:],
                                     start=(b == 0), stop=(b == nb - 1))

                # -- stats allreduce --
                arst = rp.tile([1, 2 * D], f32, tag="arst")
                nc.vector.tensor_copy(arst[:, 0:D], s1ps[:])
                nc.vector.tensor_copy(arst[:, D:2 * D], s2ps[:])
                nc.sync.dma_start(ar_in[:], arst[:])
                nc.gpsimd.collective_compute(
                    "AllReduce", OP.add, ins=[ar_in[:].opt()],
                    outs=[ar_out[:].opt()], replica_groups=rg)
                stat = rp.tile([1, 2 * D], f32, tag="stat")
                nc.sync.dma_start(stat[:], ar_out[:])

                # -- BN coefficients (rows on partition 0) --
                mu = rp.tile([1, D], f32, tag="mu")
                var = rp.tile([1, D], f32, tag="var")
                rs = rp.tile([1, D], f32, tag="rs")
                arow = rp.tile([1, D], f32, tag="arow")
                brow = rp.tile([1, D], f32, tag="brow")
                nc.vector.tensor_scalar_mul(mu[:], stat[:, 0:D], 1.0 / n_nodes)
                nc.vector.tensor_scalar_mul(var[:], stat[:, D:2 * D], 1.0 / n_nodes)
                tmp = rp.tile([1, D], f32, tag="musq")
                nc.vector.tensor_tensor(tmp[:], mu[:], mu[:], op=OP.mult)
                nc.vector.tensor_tensor(var[:], var[:], tmp[:], op=OP.subtract)
                nc.vector.tensor_scalar_add(var[:], var[:], EPS)
                nc.scalar.activation(rs[:], var[:], AF.Sqrt)
                nc.vector.reciprocal(rs[:], rs[:])
                nc.vector.tensor_tensor(arow[:], gam[:], rs[:], op=OP.mult)
                nc.vector.tensor_tensor(tmp[:], mu[:], arow[:], op=OP.mult)
                nc.vector.tensor_tensor(brow[:], bet[:], tmp[:], op=OP.subtract)
                afps = psp.tile([P, D], f32, tag="ps", space="PSUM")
                nc.tensor.matmul(afps[:], ones_row[:], arow[:], start=True, stop=True)
                nc.vector.tensor_copy(a_full[:], afps[:])
                bfps = psp.tile([P, D], f32, tag="ps", space="PSUM")
                nc.tensor.matmul(bfps[:], ones_row[:], brow[:], start=True, stop=True)
                nc.vector.tensor_copy(b_full[:], bfps[:])

                # -- apply BN + relu (+ norm or mask) --
                arep = a_full[:, None, :].to_broadcast([P, nb, D])
                brep = b_full[:, None, :].to_broadcast([P, nb, D])
                if lyr < L - 1:
                    nc.vector.tensor_tensor(s_all[:], s_all[:], arep, op=OP.mult)
                    nc.vector.tensor_tensor(s_all[:], s_all[:], brep, op=OP.add)
                    nc.vector.scalar_tensor_tensor(
                        u_all[:], s_all[:], 0.0,
                        norm[:, :, None].to_broadcast([P, nb, D]),
                        op0=OP.max, op1=OP.mult)
                    nc.sync.dma_start(
                        ag_in[:].rearrange("(p b) d -> p b d", p=P), u_all[:])
                    nc.gpsimd.collective_compute(
                        "AllGather", OP.bypass, ins=[ag_in[:].opt()],
                        outs=[ag_out[:].opt()], replica_groups=rg)
                else:
                    nc.vector.tensor_tensor(s_all[:], s_all[:], arep, op=OP.mult)
                    nc.vector.tensor_tensor(s_all[:], s_all[:], brep, op=OP.add)
                    nc.vector.scalar_tensor_tensor(
                        s_all[:], s_all[:], 0.0,
                        mask_s[:, :, None].to_broadcast([P, nb, D]),
                        op0=OP.max, op1=OP.mult)

            # ---- epilogue: x = h4 @ W1 + b1 (feature-major), BN2, W2 ----
            GRP = 4
            for g in range(0, nb, GRP):
                gw = min(GRP, nb - g)
                tr4 = tp.tile([P, GRP * D], f32, tag="tr4")
                for k in range(gw):
                    trp = psp.tile([P, D], f32, tag="ps", space="PSUM")
                    nc.tensor.transpose(out=trp[:], in_=s_all[:, g + k, :],
                                        identity=ident[:])
                    nc.vector.tensor_copy(tr4[:, k * D:(k + 1) * D], trp[:])
                xps = pspw.tile([P, GRP * D], f32, tag="psw", space="PSUM")
                nc.tensor.matmul(xps[:, :gw * D], W1_s[:], tr4[:, :gw * D],
                                 start=True, stop=True)
                nc.vector.tensor_scalar_add(
                    xT_all[:, g:g + gw, :].rearrange("p b d -> p (b d)"),
                    xps[:, :gw * D], b1c[:])
            st2 = rp.tile([D, 2], f32, tag="st2")
            nc.vector.tensor_reduce(st2[:, 0:1], xT_all[:],
                                    axis=mybir.AxisListType.XY, op=OP.add)
            nc.scalar.activation(s_all[:], xT_all[:], AF.Square)
            nc.vector.tensor_reduce(st2[:, 1:2], s_all[:],
                                    axis=mybir.AxisListType.XY, op=OP.add)
            nc.sync.dma_start(ar2_in[:], st2[:])
            nc.gpsimd.collective_compute(
                "AllReduce", OP.add, ins=[ar2_in[:].opt()],
                outs=[ar2_out[:].opt()], replica_groups=rg)
            st2g = rp.tile([D, 2], f32, tag="st2g")
            nc.sync.dma_start(st2g[:], ar2_out[:])

            # pad-row correction: pad nodes contribute b1 to S1 and b1^2 to S2
            s1c = rp.tile([D, 1], f32, tag="s1c")
            s2c = rp.tile([D, 1], f32, tag="s2c")
            b1sq = rp.tile([D, 1], f32, tag="b1sq")
            nc.vector.scalar_tensor_tensor(s1c[:], b1c[:], -padcnt, st2g[:, 0:1],
                                           op0=OP.mult, op1=OP.add)
            nc.scalar.activation(b1sq[:], b1c[:], AF.Square)
            nc.vector.scalar_tensor_tensor(s2c[:], b1sq[:], -padcnt, st2g[:, 1:2],
                                           op0=OP.mult, op1=OP.add)

            mu2 = rp.tile([D, 1], f32, tag="mu2")
            var2 = rp.tile([D, 1], f32, tag="var2")
            rs2 = rp.tile([D, 1], f32, tag="rs2")
            a2 = rp.tile([D, 1], f32, tag="a2")
            bb2 = rp.tile([D, 1], f32, tag="bb2")
            t2 = rp.tile([D, 1], f32, tag="t2")
            nc.vector.tensor_scalar_mul(mu2[:], s1c[:], 1.0 / n_nodes)
            nc.vector.tensor_scalar_mul(var2[:], s2c[:], 1.0 / n_nodes)
            nc.vector.tensor_tensor(t2[:], mu2[:], mu2[:], op=OP.mult)
            nc.vector.tensor_tensor(var2[:], var2[:], t2[:], op=OP.subtract)
            nc.vector.tensor_scalar_add(var2[:], var2[:], EPS)
            nc.scalar.activation(rs2[:], var2[:], AF.Sqrt)
            nc.vector.reciprocal(rs2[:], rs2[:])
            nc.vector.tensor_tensor(a2[:], g2c[:], rs2[:], op=OP.mult)
            nc.vector.tensor_tensor(t2[:], mu2[:], a2[:], op=OP.mult)
            nc.vector.tensor_tensor(bb2[:], b2c[:], t2[:], op=OP.subtract)

            # xT_bn = xT * a2 + bb2 (per-partition scalars), then W2
            nc.vector.tensor_scalar(xT_all[:], xT_all[:], a2[:], bb2[:],
                                    op0=OP.mult, op1=OP.add)
            for g in range(0, nb, GRP):
                gw = min(GRP, nb - g)
                yps = pspw.tile([c_dim, GRP * D], f32, tag="psw", space="PSUM")
                nc.tensor.matmul(yps[:, :gw * D], W2_s[:],
                                 xT_all[:, g:g + gw, :].rearrange("p b d -> p (b d)"),
                                 start=True, stop=True)
                yst = tp.tile([c_dim, GRP * D], f32, tag="yst")
                nc.vector.tensor_scalar_add(yst[:, :gw * D], yps[:, :gw * D], boc[:])
                nc.sync.dma_start(yT[:, g * D:(g + gw) * D], yst[:, :gw * D])

    nc.compile()
    return nc


# ----------------------------------------------------------------------------
# entry point
# ----------------------------------------------------------------------------

def kernel(h, src, dst, bn_gamma, bn_beta, W1, b1, bn2_gamma, bn2_beta, W2, b2,
           _return_perf=None):
    from concourse.bass_utils import run_bass_kernel_spmd

    h = np.asarray(h, np.float32)
    src = np.asarray(src)
    dst = np.asarray(dst)
    n_nodes = h.shape[0]
    c_dim = np.asarray(W2).shape[1]

    key = (n_nodes, src.tobytes()[:4096], dst.tobytes()[:4096], len(src))
    if key in _cache:
        meta, per_rank, nc = _cache[key]
    else:
        meta, per_rank = _preprocess(src, dst, n_nodes)
        nc = _build(meta, c_dim)
        _cache.clear()
        _cache[key] = (meta, per_rank, nc)

    shard = meta["shard"]
    in_maps = []
    for r in range(NCORES):
        pr = per_rank[r]
        hsh = np.zeros((shard, D), np.float32)
        nrows = pr["h_rows"]
        if nrows > 0:
            hsh[:nrows] = h[r * shard: r * shard + nrows]
        in_maps.append({
            "h_in": hsh,
            "M_in": pr["M"],
            "il_in": pr["idx_lo"],
            "ih_in": pr["idx_hi"],
            "mask_in": pr["mask"],
            "deg_in": pr["deg"],
            "gam_in": np.asarray(bn_gamma, np.float32).reshape(1, D),
            "bet_in": np.asarray(bn_beta, np.float32).reshape(1, D),
            "W1_in": np.asarray(W1, np.float32),
            "b1c_in": np.asarray(b1, np.float32).reshape(D, 1),
            "g2c_in": np.asarray(bn2_gamma, np.float32).reshape(D, 1),
            "b2c_in": np.asarray(bn2_beta, np.float32).reshape(D, 1),
            "W2_in": np.asarray(W2, np.float32),
            "bo_in": np.asarray(b2, np.float32).reshape(c_dim, 1),
        })

    kw = {}
    if _return_perf:
        kw = dict(trace=True, tmpdir=_return_perf)
    res = run_bass_kernel_spmd(nc, in_maps, core_ids=list(range(NCORES)), **kw)

    out = np.empty((n_nodes, c_dim), np.float32)
    for r in range(NCORES):
        nrows = min(shard, max(0, n_nodes - r * shard))
        if nrows > 0:
            out[r * shard: r * shard + nrows] = res.results[r]["yT"].T[:nrows]
    if _return_perf:
        kernel.last_exec_time_ns = res.exec_time_ns
    return out

